# revision 14
# baseline (speedup 1.0000x reference)
"""Trainium2 Bass kernel for nn_MeanSpikeClassifier.

Problem: spike_trace [16, 256, 2304] f32 ->
  per-sample v = mean(spike[127:], axis=time)  [2304]
  connected components of {|v_i - v_j| <= 0.1} (threshold graph on a line)
  one-hot masks[rank(component(pixel)), pixel], components ranked by their
  smallest pixel index; output [16, 2304, 48, 48] f32 zero-padded.

Algorithm (sort-free, cell-grid):
  Work with u = v + 1 in (1, 2). Overlay 128 value-cells of width h = 1/128
  (bf16-exact grid in [1, 2)). h <= EPS=0.1, so every cell lies entirely
  inside one component => number of components C <= 128, and components are
  runs of occupied cells chained by (min_next - max_prev <= EPS).
  Per-cell aggregates (masked max reduces): cmax, cmin, min pixel index.
  Cell-level chain logic on [1,128] rows (scans), then the output block
  out[r, i] = A[c(i), r] via one 128x128 @ 128x2304 matmul, where
  A[c, r] = occ[c] * (cellrank[c] == r).
  Only output rows 0..127 can be nonzero; rows 128.. stay zero via the
  run_bass_kernel_spmd pre-zeroed-output contract.

Sharding: pure batch data parallelism, 2 samples per core across 8 cores.
"""
import sys

if "/opt/trn_rl_repo" not in sys.path:
    sys.path.insert(0, "/opt/trn_rl_repo")

import numpy as np

import concourse.bass as bass
import concourse.mybir as mybir
from concourse.tile import TileContext
from concourse.bass_utils import run_bass_kernel_spmd

F32 = mybir.dt.float32
BF16 = mybir.dt.bfloat16
Op = mybir.AluOpType
AF = mybir.ActivationFunctionType

B, T, N = 16, 256, 2304
H = W = 48
NT = 129            # timesteps averaged: t = 127..255
S = 2               # samples per core
NCORES = 8
EPS = np.float32(1.0 - 0.9)
NC_CELLS = 128
BIG = 4096.0
CHUNKS = [(0, 512), (512, 512), (1024, 512), (1536, 512), (2048, 256)]


def build_module(stages=99):
    nc = bass.Bass()
    x = nc.declare_dram_parameter("x", [S, NT, N], F32, isOutput=False)
    cgrid = nc.declare_dram_parameter("cgrid", [128, 2], F32, isOutput=False)
    onescolf = nc.declare_dram_parameter("onescolf", [128, 1], F32, isOutput=False)
    onesrow = nc.declare_dram_parameter("onesrow", [1, 128], BF16, isOutput=False)
    onesrowf = nc.declare_dram_parameter("onesrowf", [1, 128], F32, isOutput=False)
    iotar = nc.declare_dram_parameter("iotar", [128, 128], BF16, isOutput=False)
    revio = nc.declare_dram_parameter("revio", [128, N], F32, isOutput=False)
    ident = nc.declare_dram_parameter("ident", [128, 128], F32, isOutput=False)
    y = nc.declare_dram_parameter("y", [S, N, N], F32, isOutput=True)

    with TileContext(nc) as tc:
        with (
            tc.tile_pool(name="consts", bufs=1) as cpool,
            tc.tile_pool(name="data", bufs=2) as dpool,
            tc.tile_pool(name="work", bufs=2) as wpool,
            tc.tile_pool(name="scrp", bufs=2) as spool,
            tc.tile_pool(name="rows", bufs=2) as rpool,
            tc.tile_pool(name="cols", bufs=2) as kpool,
            tc.tile_pool(name="outs", bufs=3) as opool,
            tc.tile_pool(name="pmid", bufs=2, space="PSUM") as pmid,
            tc.tile_pool(name="pout", bufs=2, space="PSUM") as pout,
            tc.tile_pool(name="ptiny", bufs=1, space="PSUM") as ptiny,
        ):
            c_grid = cpool.tile_from(cgrid[:, :])
            c_onescolf = cpool.tile_from(onescolf[:, :])
            c_onesrow = cpool.tile_from(onesrow[:, :])
            c_onesrowf = cpool.tile_from(onesrowf[:, :])
            c_iotar = cpool.tile_from(iotar[:, :])
            c_revio = cpool.tile_from(revio[:, :])
            c_ident = cpool.tile_from(ident[:, :])

            for s in range(S):
                # ---- load spike slice (f32, HWDGE; SWDGE waits overflow
                # the matmul sync-wait slots in walrus codegen)
                xt = dpool.tile([128, N], F32)
                nc.sync.dma_start(xt[:], x[s, 0:128, :])
                xr = dpool.tile([1, N], F32)
                nc.sync.dma_start(xr[:], x[s, 128:129, :])

                # ---- u_row = mean/129 + 1  (bf16 [1, N])
                u_row = rpool.tile([1, N], BF16)
                for o, w in CHUNKS:
                    pmt = pmid.tile([128, 512], F32, tag="mid")
                    pm = pmt[0:1, :]
                    nc.tensor.matmul(pm[0:1, 0:w], c_onescolf[:, :],
                                     xt[:, o:o + w], start=True, stop=False)
                    nc.tensor.matmul(pm[0:1, 0:w], c_onescolf[0:1, 0:1],
                                     xr[0:1, o:o + w], start=False, stop=True)
                    nc.scalar.activation(u_row[0:1, o:o + w], pm[0:1, 0:w],
                                         AF.Copy, bias=1.0, scale=1.0 / NT)

                if stages < 11:
                    continue
                # ---- broadcast u along partitions: uF [128, N] bf16
                uF = wpool.tile([128, N], BF16)
                for o, w in CHUNKS:
                    pc = pmid.tile([128, 512], F32, tag="mid")
                    nc.tensor.matmul(pc[:, 0:w], c_onesrow[:, :],
                                     u_row[0:1, o:o + w], start=True, stop=True)
                    nc.scalar.activation(uF[:, o:o + w], pc[:, 0:w], AF.Copy)

                if stages < 12:
                    continue
                # ---- cell one-hot B[c, i] = (u_i >= lo_c) - (u_i >= hi_c)
                p_hi = wpool.tile([128, N], BF16)
                nc.vector.tensor_scalar(p_hi[:], uF[:], c_grid[:, 1:2], None,
                                        Op.is_ge)
                bcell = wpool.tile([128, N], BF16)
                nc.vector.scalar_tensor_tensor(bcell[:], uF[:], c_grid[:, 0:1],
                                               p_hi[:], Op.is_ge, Op.subtract)
                nUF = wpool.tile([128, N], BF16)
                nc.vector.tensor_scalar(nUF[:], uF[:], -1.0, 2.0, Op.mult, Op.add)
                bf32 = spool.tile([128, N], F32, tag="bf32")
                nc.scalar.activation(bf32[:], bcell[:], AF.Copy)

                if stages < 13:
                    continue
                # ---- masked per-cell reduces -> agg cols [128, 3] f32
                # col0: cmax_u; col1: 2 - cmin_u; col2: 2305 - minidx
                agg = kpool.tile([128, 4], F32)
                scr1 = spool.tile([128, N], BF16, tag="scr1")
                nc.vector.tensor_tensor(scr1[:], bcell[:], uF[:], Op.mult)
                nc.vector.tensor_reduce(agg[:, 0:1], scr1[:],
                                        mybir.AxisListType.X, Op.max)
                scr2 = spool.tile([128, N], BF16, tag="scr2")
                nc.vector.tensor_tensor(scr2[:], bcell[:], nUF[:], Op.mult)
                nc.vector.tensor_reduce(agg[:, 1:2], scr2[:],
                                        mybir.AxisListType.X, Op.max)
                scr3 = spool.tile([128, N], F32, tag="scr3")
                nc.vector.tensor_tensor(scr3[:], bf32[:], c_revio[:, :], Op.mult)
                nc.vector.tensor_reduce(agg[:, 2:3], scr3[:],
                                        mybir.AxisListType.X, Op.max)

                if stages < 14:
                    continue
                # ---- transpose agg columns -> rows [1, 384] f32
                prow = ptiny.tile([1, 384], F32, tag="prow")
                for k in range(3):
                    nc.tensor.transpose(prow[0:1, 128 * k:128 * (k + 1)],
                                        agg[:, k:k + 1], c_ident[:, :])
                aggrow = rpool.tile([1, 384], F32)
                nc.vector.tensor_copy(aggrow[:], prow[:])
                cmax_row = aggrow[0:1, 0:128]
                nmax_row = aggrow[0:1, 128:256]
                vmax_row = aggrow[0:1, 256:384]

                if stages < 20:
                    continue
                # ---- cell-level chain logic on [1, 128] rows
                occ_row = rpool.tile([1, 128], F32)
                nc.vector.tensor_scalar(occ_row[:], cmax_row, 0.5, None, Op.is_gt)
                # exclusive running max of cmax: pm129[0] = 0
                pm129 = rpool.tile([1, 129], F32)
                nc.vector.memset(pm129[0:1, 0:1], 0.0)
                nc.vector.tensor_tensor_scan(pm129[0:1, 1:129], cmax_row,
                                             cmax_row, 0.0, Op.max, Op.bypass)
                # gap test: cmin - prevmax > EPS <=> nmax + prevmax < 2 - EPS
                s_row = rpool.tile([1, 128], F32)
                nc.vector.tensor_tensor(s_row[:], nmax_row, pm129[0:1, 0:128],
                                        Op.add)
                braw = rpool.tile([1, 128], F32)
                nc.vector.tensor_scalar(braw[:], s_row[:], float(2.0 - EPS),
                                        None, Op.is_lt)
                brk129 = rpool.tile([1, 129], F32)
                nc.vector.memset(brk129[0:1, 128:129], 0.0)
                nc.vector.tensor_tensor(brk129[0:1, 0:128], braw[:], occ_row[:],
                                        Op.mult)
                brow = brk129[0:1, 0:128]
                # min pixel index per cell (2305 = none)
                cidx_row = rpool.tile([1, 128], F32)
                nc.scalar.activation(cidx_row[:], vmax_row, AF.Copy,
                                     bias=2305.0, scale=-1.0)
                # segmented (per component) min of cidx: fwd and bwd scans
                bigb = rpool.tile([1, 128], F32)
                nc.vector.tensor_scalar(bigb[:], brow, BIG, None, Op.mult)
                fwd = rpool.tile([1, 128], F32)
                nc.vector.tensor_tensor_scan(fwd[:], bigb[:], cidx_row[:], BIG,
                                             Op.add, Op.min)
                rbig = rpool.tile([1, 128], F32)
                nc.vector.tensor_scalar(rbig[:], brk129[0:1, 1:129], BIG, None,
                                        Op.mult)
                bwd = rpool.tile([1, 128], F32)
                nc.vector.tensor_tensor_scan(bwd[0:1, ::-1], rbig[0:1, ::-1],
                                             cidx_row[0:1, ::-1], BIG,
                                             Op.add, Op.min)
                fullmin = rpool.tile([1, 128], F32)
                nc.vector.tensor_tensor(fullmin[:], fwd[:], bwd[:], Op.min)
                # w[c] = fullmin if break else BIG
                wtmp = rpool.tile([1, 128], F32)
                nc.vector.scalar_tensor_tensor(wtmp[:], fullmin[:], -BIG,
                                               brow, Op.add, Op.mult)
                wrow = rpool.tile([1, 128], F32)
                nc.vector.tensor_scalar(wrow[:], wtmp[:], BIG, None, Op.add)

                if stages < 30:
                    continue
                # ---- ranks: cellrank[c] = #{c' : w[c'] < fullmin[c]}
                pw = ptiny.tile([128, 128], F32, tag="pw")
                nc.tensor.matmul(pw[:], c_onesrowf[:, :], wrow[:],
                                 start=True, stop=True)
                pc2 = ptiny.tile([128, 2], F32, tag="pc2")
                nc.tensor.transpose(pc2[:, 0:1], fullmin[:], c_ident[0:1, 0:1])
                nc.tensor.transpose(pc2[:, 1:2], occ_row[:], c_ident[0:1, 0:1])
                cols2 = kpool.tile([128, 2], F32)
                nc.vector.tensor_copy(cols2[:], pc2[:])
                prs = kpool.tile([128, 128], F32)
                crank = kpool.tile([128, 1], F32)
                nc.vector.tensor_scalar(prs[:], pw[:], cols2[:, 0:1], None,
                                        Op.is_lt, Op.add, accum_out=crank[:])

                # ---- A[c, r] = (r == cellrank[c]) * occ[c]   bf16 [128, 128]
                amat = kpool.tile([128, 128], BF16)
                nc.vector.tensor_scalar(amat[:], c_iotar[:, :], crank[:],
                                        cols2[:, 1:2], Op.is_equal, Op.mult)

                if stages < 40:
                    continue
                # ---- out rows 0..127 = A.T @ B ; rows 128.. stay zero
                for o, w in CHUNKS:
                    po = pout.tile([128, 512], F32)
                    nc.tensor.matmul(po[:, 0:w], amat[:], bcell[:, o:o + w],
                                     start=True, stop=True)
                    osb = opool.tile([128, 512], F32)
                    nc.scalar.activation(osb[:, 0:w], po[:, 0:w], AF.Copy)
                    nc.sync.dma_start(y[s, 0:128, o:o + w], osb[:, 0:w])
    return nc


def _legalize_waits(nc, maxw=1):
    """Split multi-wait instructions into single-wait NOP carriers.

    The walrus build in this container rejects instructions carrying more
    than one semaphore wait ("Too many sync wait commands" in
    setupSyncWait). Tile freely attaches several waits to one instruction,
    so after tracing we rewrite: extra waits move onto fresh NOPs placed
    immediately before the instruction on the same engine stream. A NOP
    stalling earlier in the same stream is semantically identical, just a
    few sequencer cycles slower.
    """
    f = nc.m.functions[0]
    blocks = list(f.blocks)
    for blk in blocks:
        insts = blk.instructions
        idx = 0
        while idx < len(insts):
            inst = insts[idx]
            si = inst.sync_info
            if si is not None and si.on_wait and len(si.on_wait) > maxw:
                waits = list(si.on_wait)
                extra, keep = waits[:-maxw], waits[-maxw:]
                si.on_wait = keep
                carriers = []
                for w in extra:
                    nop = mybir.InstNoOp(
                        name=nc.get_next_instruction_name(),
                        sync_info=mybir.SyncInfo(on_wait=[w], on_update=[]),
                        engine=inst.engine,
                        bass_nofuse=True,
                        text_hint="waitsplit",
                    )
                    nc.register_instruction(nop)
                    carriers.append(nop)
                for c in reversed(carriers):
                    insts.insert(idx, c)
                idx += len(carriers)
            idx += 1
    return nc


def build_consts():
    h = 1.0 / NC_CELLS
    c = np.arange(NC_CELLS, dtype=np.float64)
    grid = np.zeros((128, 2), np.float32)
    grid[:, 0] = 1.0 + c * h
    grid[:, 1] = 1.0 + (c + 1) * h
    grid[127, 1] = 2.5  # widen last cell: never lose a pixel at the top edge
    consts = {
        "cgrid": grid,
        "onescolf": np.ones((128, 1), np.float32),
        "onesrow": np.ones((1, 128), np.float32),
        "onesrowf": np.ones((1, 128), np.float32),
        "iotar": np.tile(np.arange(128, dtype=np.float32)[None, :], (128, 1)),
        "revio": np.tile((2305.0 - np.arange(N, dtype=np.float64))
                         .astype(np.float32)[None, :], (128, 1)),
        "ident": np.eye(128, dtype=np.float32),
    }
    try:
        import ml_dtypes
        bf = ml_dtypes.bfloat16
        for k in ("onesrow", "iotar"):
            consts[k] = consts[k].astype(bf)
    except ImportError:
        import jax.numpy as jnp
        for k in ("onesrow", "iotar"):
            consts[k] = np.asarray(jnp.asarray(consts[k], jnp.bfloat16))
    return consts


_CACHE = {}


def _get_module():
    if "nc" not in _CACHE:
        _CACHE["nc"] = _legalize_waits(build_module())
        _CACHE["consts"] = build_consts()
    return _CACHE["nc"], _CACHE["consts"]


def run(spike_trace, trace=False):
    nc, consts = _get_module()
    spike = np.ascontiguousarray(np.asarray(spike_trace, dtype=np.float32))
    assert spike.shape == (B, T, N), spike.shape
    xs = spike[:, T - NT:, :]                      # [16, 129, 2304]
    in_maps = []
    for k in range(NCORES):
        m = {"x": np.ascontiguousarray(xs[S * k:S * (k + 1)])}
        m.update(consts)
        in_maps.append(m)
    res = run_bass_kernel_spmd(nc, in_maps, list(range(NCORES)), trace=trace)
    out = np.concatenate([res.results[k]["y"] for k in range(NCORES)], axis=0)
    return out.reshape(B, N, H, W), res


def kernel(spike_trace):
    out, _ = run(spike_trace, trace=False)
    return out
